# revision 6
# baseline (speedup 1.0000x reference)
"""Chamfer distance loss kernel for Trainium2 (8 NeuronCores, batch-parallel).

Math: for each batch element, d2(i,j) = |s_i|^2 + |t_j|^2 - 2 s_i.t_j.
We fold the whole distance computation into a single K=5 matmul by augmenting:
  S_aug = [sx, sy, sz, -0.5|s|^2, 1]   (5 x 4096)
  T_aug = [tx, ty, tz, 1, -0.5|t|^2]   (5 x 4096)
so (S_aug)^T @ T_aug = s.t - 0.5|s|^2 - 0.5|t|^2 = -0.5 * d2.
min_j d2 = -2 * max_j(-0.5 d2): every reduction becomes a MAX.

Per core (one batch element):
  - PE: 32x8 matmul tiles [128, 512] of -0.5*d2 into PSUM.
  - DVE: reduce_max over free axis (source-point row maxes), tensor_max
    accumulation into colmax (target-point maxes, partition-resolved).
  - PE transposes colmax blocks; DVE reduce_max finishes target maxes.
  - The 4096+4096 per-point values (-0.5*min d2) are DMAed to DRAM.
Host: clamp/scale/sqrt/mean in numpy and average the 8 batch scalars.
"""

import sys

for _p in ("/opt/trn_rl_repo", "/root/.axon_site/_ro/trn_rl_repo"):
    if _p not in sys.path:
        sys.path.insert(0, _p)

import numpy as np

import concourse.bass as bass
import concourse.bacc as bacc
import concourse.tile as tile
from concourse import mybir
from concourse.bass_utils import run_bass_kernel_spmd

FP32 = mybir.dt.float32
AX = mybir.AxisListType
ALU = mybir.AluOpType

B = 8          # batch == number of cores
N = 4096       # points per cloud
D = 3
P = 128        # partition tile (source tile size)
NT = N // P    # 32 source tiles
CH = 512       # target chunk (one PSUM bank of fp32)
NCH = N // CH  # 8 target chunks
NCORES = 8

LAST_RESULTS = None  # BassKernelResults of the most recent run (for test.py)


def _build_aug(tc, pool, psum_pool, dram_in, ident, aug, s2_row, ones_row):
    """Fill aug [5, N] from dram_in [N, 3].

    Column ordering is the "comb" permutation c = 128*a + p  <->  point 32*p + a
    (harmless: the chamfer loss is invariant to point order).
    rows 0..2 = coords, row s2_row = -0.5*|pt|^2, row ones_row = 1.0.
    """
    nc = tc.nc
    # the ones row is whatever the fill leaves untouched by the DMAs below
    nc.vector.memset(aug[:], 1.0)
    comb = pool.tile([P, P], FP32, tag="comb")
    # contiguous load: partition p holds points [32p, 32p+32) as 96 floats
    nc.sync.dma_start(comb[:, 0:96], dram_in.rearrange("(p a) d -> p (a d)", p=P))
    sq = pool.tile([P, 96], FP32, tag="sq")
    nc.scalar.square(sq[:], comb[:, 0:96])
    s2 = pool.tile([P, 32], FP32, tag="s2")
    nc.vector.tensor_reduce(
        s2[:], sq[:].rearrange("p (a d) -> p a d", d=3), axis=AX.X, op=ALU.add
    )
    nc.vector.tensor_scalar_mul(comb[:, 96:128], s2[:], -0.5)
    ps = psum_pool.tile([P, P], FP32, tag="tpose")
    nc.tensor.transpose(ps[:], comb[:], ident[:])
    ct = pool.tile([P, P], FP32, tag="ct")
    nc.scalar.copy(ct[:], ps[:])
    # gather transposed comb rows into the 5 x N augmented matrix
    # (partition-contiguous chunks only: strided-partition APs confuse the
    # tile dependency tracker)
    for a in range(32):
        nc.sync.dma_start(aug[0:3, a * P : (a + 1) * P], ct[3 * a : 3 * a + 3, :])
    nc.sync.dma_start(aug[s2_row : s2_row + 1, :], ct[96:128, :])
    del ones_row  # row left at the memset value 1.0


def _kernel_body(tc, src, tgt, ident_dram, mins_out):
    nc = tc.nc
    with (
        tc.tile_pool(name="const", bufs=1) as const_pool,
        tc.tile_pool(name="aug", bufs=1) as aug_pool,
        tc.tile_pool(name="build", bufs=2) as build_pool,
        tc.tile_pool(name="acc", bufs=1) as acc_pool,
        tc.tile_pool(name="psum", bufs=6, space="PSUM") as psum_pool,
        tc.tile_pool(name="psumt", bufs=2, space="PSUM") as psumt_pool,
    ):
        ident = const_pool.tile([P, P], FP32)
        nc.sync.dma_start(ident[:], ident_dram)

        s_aug = aug_pool.tile([5, N], FP32, tag="s_aug")
        t_aug = aug_pool.tile([5, N], FP32, tag="t_aug")
        _build_aug(tc, build_pool, psumt_pool, src, ident, s_aug, s2_row=3, ones_row=4)
        _build_aug(tc, build_pool, psumt_pool, tgt, ident, t_aug, s2_row=4, ones_row=3)

        # accumulators
        colmax = acc_pool.tile([P, N], FP32, tag="colmax")      # [src_part, tgt_col]
        rowparts = acc_pool.tile([P, NT * NCH], FP32, tag="rowparts")

        for it in range(NT):
            lhsT = s_aug[:, it * P : (it + 1) * P]
            for c in range(NCH):
                ps = psum_pool.tile([P, CH], FP32, tag="d2")
                nc.tensor.matmul(
                    ps[:], lhsT, t_aug[:, c * CH : (c + 1) * CH], start=True, stop=True
                )
                # row (source-point) max over this chunk
                col = it * NCH + c
                nc.vector.tensor_reduce(
                    rowparts[:, col : col + 1], ps[:], axis=AX.X, op=ALU.max
                )
                # column (target-point) max accumulation across source tiles
                csl = colmax[:, c * CH : (c + 1) * CH]
                if it == 0:
                    nc.scalar.copy(csl, ps[:])
                else:
                    nc.vector.tensor_max(csl, csl, ps[:])

        # finish rows: max over the 8 chunk-partials -> mins_out[:, 0:32]
        mins_sb = acc_pool.tile([P, 2 * NT], FP32, tag="mins")
        nc.vector.tensor_reduce(
            mins_sb[:, 0:NT],
            rowparts[:].rearrange("p (t c) -> p t c", c=NCH),
            axis=AX.X,
            op=ALU.max,
        )
        # finish columns: transpose 128-blocks, reduce over former partitions
        for cb in range(N // P):
            pst = psumt_pool.tile([P, P], FP32, tag="tpose")
            nc.tensor.transpose(pst[:], colmax[:, cb * P : (cb + 1) * P], ident[:])
            nc.vector.tensor_reduce(
                mins_sb[:, NT + cb : NT + cb + 1], pst[:], axis=AX.X, op=ALU.max
            )

        nc.sync.dma_start(mins_out, mins_sb[:])


_CACHE = {}


def _get_program():
    if "nc" not in _CACHE:
        nc = bacc.Bacc(
            "TRN2",
            target_bir_lowering=False,
            debug=False,
            enable_asserts=True,
            num_devices=NCORES,
        )
        src = nc.dram_tensor("src", [N, D], FP32, kind="ExternalInput")
        tgt = nc.dram_tensor("tgt", [N, D], FP32, kind="ExternalInput")
        ident = nc.dram_tensor("ident", [P, P], FP32, kind="ExternalInput")
        mins = nc.dram_tensor("mins", [P, 2 * NT], FP32, kind="ExternalOutput")
        with tile.TileContext(nc) as tc:
            _kernel_body(tc, src.ap(), tgt.ap(), ident.ap(), mins.ap())
        nc.compile()
        _CACHE["nc"] = nc
    return _CACHE["nc"]


def kernel(source: np.ndarray, target: np.ndarray) -> np.ndarray:
    global LAST_RESULTS
    import os

    source = np.ascontiguousarray(np.asarray(source, dtype=np.float32))
    target = np.ascontiguousarray(np.asarray(target, dtype=np.float32))
    assert source.shape == (B, N, D) and target.shape == (B, N, D)

    nc = _get_program()
    eye = np.eye(P, dtype=np.float32)
    in_maps = [
        {"src": source[b], "tgt": target[b], "ident": eye} for b in range(B)
    ]
    trace = os.environ.get("CHAMFER_TRACE", "0") == "1"
    tmpdir = os.environ.get("CHAMFER_TMPDIR") or None
    res = run_bass_kernel_spmd(
        nc, in_maps, core_ids=list(range(NCORES)), trace=trace, tmpdir=tmpdir
    )
    LAST_RESULTS = res

    # host epilogue: mins holds -0.5 * min d2 (as a max); clamp, scale, sqrt, mean
    loss = 0.0
    for b in range(B):
        m = res.results[b]["mins"].astype(np.float64)
        d2 = np.maximum(-2.0 * m, 0.0)
        dist = np.sqrt(d2)
        loss += dist[:, :NT].mean() + dist[:, NT:].mean()
    loss /= B
    return np.float32(loss)


# revision 10
# speedup vs baseline: 1.1308x; 1.1308x over previous
"""Chamfer distance loss kernel for Trainium2 (8 NeuronCores, batch-parallel).

Math: for each batch element, d2(i,j) = |s_i|^2 + |t_j|^2 - 2 s_i.t_j.
The whole distance computation folds into K=5 matmuls via augmentation:
  S_aug = [sx, sy, sz, -0.5|s|^2, 1]   (5 x 4096)
  T_aug = [tx, ty, tz, 1, -0.5|t|^2]   (5 x 4096)
so (S_aug)^T @ T_aug = s.t - 0.5|s|^2 - 0.5|t|^2 = -0.5 * d2.
min_j d2 = -2 * max_j(-0.5 d2): every reduction becomes a MAX.

fp32 matmuls are compiler-split ~4x (fp32-high emulation), so instead each
augmented matrix is kept as an fp16 hi/lo pair (x ~= hi + lo, 22-bit
effective mantissa) and each distance tile is computed with 3 accumulated
fp16 matmuls: hi.hi + hi.lo + lo.hi (the lo.lo term is ~2^-22 and dropped).

Per core (one batch element):
  - PE: per source tile, 24 fp16 matmuls into 4 two-bank PSUM tiles [128,1024].
  - DVE: reduce_max over PSUM (pure fp32 row/source maxes, chunk-partials),
    tensor_max in fp16 at 2x mode for the column/target accumulation.
  - ACT: evacuates each PSUM tile to fp16 SBUF (feeds the 2x colmax pass).
  - PE transposes colmax blocks; DVE reduce_max finishes target maxes.
  - The 4096+4096 per-point values (-0.5*min d2) are DMAed to DRAM.
Host: clamp/scale/sqrt/mean in numpy and average the 8 batch scalars.
"""

import sys

for _p in ("/opt/trn_rl_repo", "/root/.axon_site/_ro/trn_rl_repo"):
    if _p not in sys.path:
        sys.path.insert(0, _p)

import numpy as np

import concourse.bass as bass
import concourse.bacc as bacc
import concourse.tile as tile
from concourse import mybir
from concourse.bass_utils import run_bass_kernel_spmd

FP32 = mybir.dt.float32
FP16 = mybir.dt.float16
AX = mybir.AxisListType
ALU = mybir.AluOpType

B = 8          # batch == number of cores
N = 4096       # points per cloud
D = 3
P = 128        # partition tile (source tile size)
NT = N // P    # 32 source tiles
CH = 512       # one PSUM bank of fp32
GRP = 1024     # two banks per PSUM tile
NG = N // GRP  # 4 groups per source tile
NCORES = 8

LAST_RESULTS = None  # BassKernelResults of the most recent run (for test.py)


def _build_aug(tc, pool, psum_pool, dram_in, ident16, aug_hi, aug_lo, s2_row):
    """Fill the fp16 hi/lo pair aug_hi/aug_lo [5, N] from dram_in [N, 3].

    Column ordering is the "comb" permutation c = 128*a + p <-> point 32*p + a
    (harmless: the chamfer loss is invariant to point order).
    rows 0..2 = coords, row s2_row = -0.5*|pt|^2, remaining row = 1.0.
    """
    nc = tc.nc
    # rows not overwritten below stay at the fill value: 1.0 (hi), 0.0 (lo)
    nc.vector.memset(aug_hi[:], 1.0)
    nc.vector.memset(aug_lo[:], 0.0)

    comb = pool.tile([P, P], FP32, tag="comb")
    # contiguous load: partition p holds points [32p, 32p+32) as 96 floats
    nc.sync.dma_start(comb[:, 0:96], dram_in.rearrange("(p a) d -> p (a d)", p=P))
    sq = pool.tile([P, 96], FP32, tag="sq")
    nc.scalar.square(sq[:], comb[:, 0:96])
    s2 = pool.tile([P, 32], FP32, tag="s2")
    nc.vector.tensor_reduce(
        s2[:], sq[:].rearrange("p (a d) -> p a d", d=3), axis=AX.X, op=ALU.add
    )
    nc.vector.tensor_scalar_mul(comb[:, 96:128], s2[:], -0.5)

    comb_hi = pool.tile([P, P], FP16, tag="comb_hi")
    nc.vector.tensor_copy(comb_hi[:], comb[:])
    comb_lo = pool.tile([P, P], FP16, tag="comb_lo")
    nc.vector.tensor_sub(comb_lo[:], comb[:], comb_hi[:])

    for part, aug in ((comb_hi, aug_hi), (comb_lo, aug_lo)):
        ps = psum_pool.tile([P, P], FP16, tag="tpose")
        nc.tensor.transpose(ps[:], part[:], ident16[:])
        ct = pool.tile([P, P], FP16, tag="ct")
        nc.scalar.copy(ct[:], ps[:])
        # gather transposed comb rows into the 5 x N augmented matrix
        # (partition-contiguous chunks only: strided-partition APs confuse
        # the tile dependency tracker)
        for a in range(32):
            nc.sync.dma_start(aug[0:3, a * P : (a + 1) * P], ct[3 * a : 3 * a + 3, :])
        nc.sync.dma_start(aug[s2_row : s2_row + 1, :], ct[96:128, :])


def _kernel_body(tc, src, tgt, ident_dram, mins_out):
    nc = tc.nc
    with (
        tc.tile_pool(name="const", bufs=1) as const_pool,
        tc.tile_pool(name="aug", bufs=1) as aug_pool,
        tc.tile_pool(name="build", bufs=2) as build_pool,
        tc.tile_pool(name="acc", bufs=1) as acc_pool,
        tc.tile_pool(name="evac", bufs=8) as evac_pool,
        tc.tile_pool(name="psum", bufs=3, space="PSUM") as psum_pool,
        tc.tile_pool(name="psumt", bufs=2, space="PSUM") as psumt_pool,
    ):
        ident = const_pool.tile([P, P], FP32)
        nc.sync.dma_start(ident[:], ident_dram)
        ident16 = const_pool.tile([P, P], FP16)
        nc.vector.tensor_copy(ident16[:], ident[:])

        s_hi = aug_pool.tile([5, N], FP16, tag="s_hi")
        s_lo = aug_pool.tile([5, N], FP16, tag="s_lo")
        t_hi = aug_pool.tile([5, N], FP16, tag="t_hi")
        t_lo = aug_pool.tile([5, N], FP16, tag="t_lo")
        _build_aug(tc, build_pool, psumt_pool, src, ident16, s_hi, s_lo, s2_row=3)
        _build_aug(tc, build_pool, psumt_pool, tgt, ident16, t_hi, t_lo, s2_row=4)

        # accumulators
        colmax = acc_pool.tile([P, N], FP16, tag="colmax")      # [src_part, tgt_col]
        rowparts = acc_pool.tile([P, NT * NG], FP32, tag="rowparts")
        mins_sb = acc_pool.tile([P, 2 * NT], FP32, tag="mins")

        for it in range(NT):
            lhs_hi = s_hi[:, it * P : (it + 1) * P]
            lhs_lo = s_lo[:, it * P : (it + 1) * P]
            pss = [
                psum_pool.tile([P, GRP], FP32, tag="d2", name=f"d2_{it}_{g}")
                for g in range(NG)
            ]
            # weight-grouped emit order: all hi-weight matmuls, then lo-weight
            for g in range(NG):
                for j in range(2):
                    c = 2 * g + j
                    nc.tensor.matmul(
                        pss[g][:, j * CH : (j + 1) * CH],
                        lhs_hi,
                        t_hi[:, c * CH : (c + 1) * CH],
                        start=True,
                        stop=False,
                    )
            for g in range(NG):
                for j in range(2):
                    c = 2 * g + j
                    nc.tensor.matmul(
                        pss[g][:, j * CH : (j + 1) * CH],
                        lhs_hi,
                        t_lo[:, c * CH : (c + 1) * CH],
                        start=False,
                        stop=False,
                    )
            for g in range(NG):
                for j in range(2):
                    c = 2 * g + j
                    nc.tensor.matmul(
                        pss[g][:, j * CH : (j + 1) * CH],
                        lhs_lo,
                        t_hi[:, c * CH : (c + 1) * CH],
                        start=False,
                        stop=True,
                    )
            for g in range(NG):
                # pure row (source-point) max for this 1024-column group
                nc.vector.tensor_reduce(
                    rowparts[:, it * NG + g : it * NG + g + 1],
                    pss[g][:].rearrange("p (j c) -> p j c", j=2),
                    axis=AX.XY,
                    op=ALU.max,
                )
                # evacuate to fp16 so the colmax pass runs at DVE 2x
                e16 = evac_pool.tile([P, GRP], FP16, tag="e16")
                nc.scalar.copy(e16[:], pss[g][:])
                csl = colmax[:, g * GRP : (g + 1) * GRP]
                if it == 0:
                    nc.vector.tensor_copy(csl, e16[:])
                else:
                    nc.vector.tensor_max(csl, csl, e16[:])

        # finish rows: max over the 4 group-partials -> mins_sb[:, 0:32]
        nc.vector.tensor_reduce(
            mins_sb[:, 0:NT],
            rowparts[:].rearrange("p (t g) -> p t g", g=NG),
            axis=AX.X,
            op=ALU.max,
        )
        # finish columns: transpose 128-blocks, reduce over former partitions
        for cb in range(N // P):
            pst = psumt_pool.tile([P, P], FP16, tag="tpose")
            nc.tensor.transpose(pst[:], colmax[:, cb * P : (cb + 1) * P], ident16[:])
            nc.vector.tensor_reduce(
                mins_sb[:, NT + cb : NT + cb + 1], pst[:], axis=AX.X, op=ALU.max
            )

        nc.sync.dma_start(mins_out, mins_sb[:])


_CACHE = {}


def _get_program():
    if "nc" not in _CACHE:
        nc = bacc.Bacc(
            "TRN2",
            target_bir_lowering=False,
            debug=False,
            enable_asserts=True,
            num_devices=NCORES,
        )
        src = nc.dram_tensor("src", [N, D], FP32, kind="ExternalInput")
        tgt = nc.dram_tensor("tgt", [N, D], FP32, kind="ExternalInput")
        ident = nc.dram_tensor("ident", [P, P], FP32, kind="ExternalInput")
        mins = nc.dram_tensor("mins", [P, 2 * NT], FP32, kind="ExternalOutput")
        with tile.TileContext(nc) as tc:
            _kernel_body(tc, src.ap(), tgt.ap(), ident.ap(), mins.ap())
        nc.compile()
        _CACHE["nc"] = nc
    return _CACHE["nc"]


def kernel(source: np.ndarray, target: np.ndarray) -> np.ndarray:
    global LAST_RESULTS
    import os

    source = np.ascontiguousarray(np.asarray(source, dtype=np.float32))
    target = np.ascontiguousarray(np.asarray(target, dtype=np.float32))
    assert source.shape == (B, N, D) and target.shape == (B, N, D)

    nc = _get_program()
    eye = np.eye(P, dtype=np.float32)
    in_maps = [
        {"src": source[b], "tgt": target[b], "ident": eye} for b in range(B)
    ]
    trace = os.environ.get("CHAMFER_TRACE", "0") == "1"
    tmpdir = os.environ.get("CHAMFER_TMPDIR") or None
    res = run_bass_kernel_spmd(
        nc, in_maps, core_ids=list(range(NCORES)), trace=trace, tmpdir=tmpdir
    )
    LAST_RESULTS = res

    # host epilogue: mins holds -0.5 * min d2 (as a max); clamp, scale, sqrt, mean
    loss = 0.0
    for b in range(B):
        m = res.results[b]["mins"].astype(np.float64)
        d2 = np.maximum(-2.0 * m, 0.0)
        dist = np.sqrt(d2)
        loss += dist[:, :NT].mean() + dist[:, NT:].mean()
    loss /= B
    return np.float32(loss)


# revision 17
# speedup vs baseline: 1.9261x; 1.7032x over previous
"""Chamfer distance loss kernel for Trainium2 (8 NeuronCores, batch-parallel).

Math: for each batch element, d2(i,j) = |s_i|^2 + |t_j|^2 - 2 s_i.t_j.
The whole distance computation folds into K=5 matmuls via augmentation:
  S_aug = [sx, sy, sz, -0.5|s|^2, 1]   (5 x 4096)
  T_aug = [tx, ty, tz, 1, -0.5|t|^2]   (5 x 4096)
so (S_aug)^T @ T_aug = s.t - 0.5|s|^2 - 0.5|t|^2 = -0.5 * d2.
min_j d2 = -2 * max_j(-0.5 d2): every reduction becomes a MAX.

fp32 matmuls are compiler-split ~4x (fp32-high emulation), so each augmented
matrix is kept as an fp16 hi/lo pair (x ~= hi + lo, 22-bit effective
mantissa) and each distance tile is computed with 3 accumulated fp16
matmuls: hi.hi + hi.lo + lo.hi (the lo.lo term is ~2^-22 and dropped).

K=5 matmuls never warm the PE activity monitor (the clock stays at 1.2 GHz)
and use 5/128 of the array, so source tiles are processed 4 at a time in
separate 32-row groups of the PE array via tile_position: the augmented
matrices are replicated at partition bases {0, 32, 64, 96} and 4 matmuls
stream concurrently, quartering the effective matmul cost.

Per core (one batch element):
  - PE: per source-tile quad, 24 row-packed fp16 matmuls into 4 two-bank
    PSUM tiles [128, 1024].
  - ACT: evacuates each PSUM tile to fp16 SBUF (the only PSUM reader).
  - DVE (all fp16 SBUF at 2x mode): tensor_max fold chains for both the
    row/source maxes (per-tile acc, one 1x reduce per source tile) and the
    column/target maxes (colmax accumulator).
  - PE transposes colmax blocks; DVE reduce_max finishes target maxes.
  - The 4096+4096 per-point values (-0.5*min d2) are DMAed to DRAM.
Host: clamp/scale/sqrt/mean in numpy and average the 8 batch scalars.
"""

import sys

for _p in ("/opt/trn_rl_repo", "/root/.axon_site/_ro/trn_rl_repo"):
    if _p not in sys.path:
        sys.path.insert(0, _p)

import numpy as np

import concourse.bass as bass
import concourse.bacc as bacc
import concourse.tile as tile
from concourse import mybir
from concourse.bass_utils import run_bass_kernel_spmd

FP32 = mybir.dt.float32
FP16 = mybir.dt.float16
AX = mybir.AxisListType
ALU = mybir.AluOpType

B = 8          # batch == number of cores
N = 4096       # points per cloud
D = 3
P = 128        # partition tile (source tile size)
NT = N // P    # 32 source tiles
CH = 512       # one PSUM bank of fp32
GRP = 1024     # two banks per PSUM tile
NG = N // GRP  # 4 column groups
NQ = NT // 4   # 8 source-tile quads
NCORES = 8

LAST_RESULTS = None  # BassKernelResults of the most recent run (for test.py)


def _build_aug(tc, pool, dram_nat, dram_t, aux, aug_hi, aug_lo,
               s2_row, ones_row, pfx, deng):
    """Build rows 0..4 (partition base 0) of the fp16 hi/lo pair
    aug_hi/aug_lo [128, N] from dram_nat [N, 3] and dram_t [3, N]
    (the same points, host-transposed — pure layout).

    rows 0..2 = coords, s2_row = -0.5*|pt|^2, ones_row = 1.0 (hi) / 0.0 (lo).
    """
    nc = tc.nc
    # coords: load [3, N] fp32, split hi/lo straight into the aug rows
    ct32 = pool.tile([3, N], FP32, tag=f"ct32_{pfx}", name=f"ct32_{pfx}")
    deng.dma_start(ct32[:], dram_t)
    nc.scalar.copy(aug_hi[0:3, :], ct32[:])
    nc.vector.tensor_sub(aug_lo[0:3, :], ct32[:], aug_hi[0:3, :])

    # -0.5|pt|^2 in the wide layout: partition p holds points [32p, 32p+32)
    comb = pool.tile([P, 96], FP32, tag=f"comb_{pfx}", name=f"comb_{pfx}")
    deng.dma_start(comb[:], dram_nat.rearrange("(p a) d -> p (a d)", p=P))
    sq = pool.tile([P, 96], FP32, tag=f"sq_{pfx}", name=f"sq_{pfx}")
    nc.scalar.square(sq[:], comb[:])
    s2 = pool.tile([P, 32], FP32, tag=f"s2_{pfx}", name=f"s2_{pfx}")
    nc.vector.tensor_reduce(
        s2[:], sq[:].rearrange("p (a d) -> p a d", d=3), axis=AX.X, op=ALU.add
    )
    nc.vector.tensor_scalar_mul(s2[:], s2[:], -0.5)
    s2h = pool.tile([P, 32], FP16, tag=f"s2h_{pfx}", name=f"s2h_{pfx}")
    nc.vector.tensor_copy(s2h[:], s2[:])
    s2l = pool.tile([P, 32], FP16, tag=f"s2l_{pfx}", name=f"s2l_{pfx}")
    nc.vector.tensor_sub(s2l[:], s2[:], s2h[:])
    # scatter [128, 32] -> [1, N]: iteration order (p, a) matches j = 32p + a
    deng.dma_start(aug_hi[s2_row : s2_row + 1, :], s2h[:])
    deng.dma_start(aug_lo[s2_row : s2_row + 1, :], s2l[:])
    # ones row: 1.0 for the hi part, 0.0 for the lo part
    deng.dma_start(aug_hi[ones_row : ones_row + 1, :], aux[0:1, :])
    deng.dma_start(aug_lo[ones_row : ones_row + 1, :], aux[1:2, :])


def _kernel_body(tc, src, tgt, src_t, tgt_t, ident_dram, aux_dram, mins_out):
    nc = tc.nc
    with (
        tc.tile_pool(name="const", bufs=1) as const_pool,
        tc.tile_pool(name="aug", bufs=1) as aug_pool,
        tc.tile_pool(name="build", bufs=1) as build_pool,
        tc.tile_pool(name="acc", bufs=1) as acc_pool,
        tc.tile_pool(name="accq", bufs=2) as accq_pool,
        tc.tile_pool(name="evac", bufs=2) as evac_pool,
    ):
        ident16 = const_pool.tile([P, P], FP16)
        nc.sync.dma_start(ident16[:], ident_dram)
        aux = const_pool.tile([2, N], FP16)
        nc.sync.dma_start(aux[:], aux_dram)

        s_hi = aug_pool.tile([P, N], FP16, tag="s_hi")
        s_lo = aug_pool.tile([P, N], FP16, tag="s_lo")
        t_hi = aug_pool.tile([P, N], FP16, tag="t_hi")
        t_lo = aug_pool.tile([P, N], FP16, tag="t_lo")

        _build_aug(tc, build_pool, src, src_t, aux, s_hi, s_lo,
                   s2_row=3, ones_row=4, pfx="s", deng=nc.sync)
        _build_aug(tc, build_pool, tgt, tgt_t, aux, t_hi, t_lo,
                   s2_row=4, ones_row=3, pfx="t", deng=nc.scalar)

        # replicate rows 0..4 at partition bases 32/64/96 for row-group packing
        engs = [nc.sync, nc.scalar, nc.gpsimd]
        for ti, t in enumerate((s_hi, s_lo, t_hi, t_lo)):
            for ri, base in enumerate((32, 64, 96)):
                engs[(ti + ri) % 3].dma_start(t[base : base + 5, :], t[0:5, :])

        # accumulators
        colmax = acc_pool.tile([P, N], FP16, tag="colmax")      # [src_part, tgt_col]
        mins_sb = acc_pool.tile([P, 2 * NT], FP32, tag="mins")

        with tc.tile_pool(name="psum", bufs=1, space="PSUM") as psum_pool:
            for iq in range(NQ):
                accs = [
                    accq_pool.tile([P, GRP], FP16, tag=f"acc{q}", name=f"acc_{iq}_{q}")
                    for q in range(4)
                ]
                for g in range(NG):
                    pss = [
                        psum_pool.tile([P, GRP], FP32, tag=f"d2_{q}",
                                       name=f"d2_{iq}_{g}_{q}")
                        for q in range(4)
                    ]
                    for p_i, (w, r) in enumerate(
                        ((s_hi, t_hi), (s_hi, t_lo), (s_lo, t_hi))
                    ):
                        for j in range(2):
                            c = 2 * g + j
                            for q in range(4):
                                it = iq * 4 + q
                                nc.tensor.matmul(
                                    pss[q][:, j * CH : (j + 1) * CH],
                                    w[32 * q : 32 * q + 5, it * P : (it + 1) * P],
                                    r[32 * q : 32 * q + 5, c * CH : (c + 1) * CH],
                                    start=(p_i == 0),
                                    stop=(p_i == 2),
                                    tile_position=(32 * q, 0),
                                )
                    for q in range(4):
                        e16 = evac_pool.tile([P, GRP], FP16, tag=f"e16_{q}",
                                             name=f"e16_{iq}_{g}_{q}")
                        nc.scalar.copy(e16[:], pss[q][:])
                        # row/source fold chain (per source tile)
                        if g == 0:
                            nc.vector.tensor_copy(accs[q][:], e16[:])
                        else:
                            nc.vector.tensor_max(accs[q][:], accs[q][:], e16[:])
                        # column/target fold chain
                        csl = colmax[:, g * GRP : (g + 1) * GRP]
                        if iq == 0 and q == 0:
                            nc.vector.tensor_copy(csl, e16[:])
                        else:
                            nc.vector.tensor_max(csl, csl, e16[:])
                for q in range(4):
                    nc.vector.tensor_reduce(
                        mins_sb[:, iq * 4 + q : iq * 4 + q + 1],
                        accs[q][:],
                        axis=AX.X,
                        op=ALU.max,
                    )

        # finish columns: transpose 128-blocks, reduce over former partitions
        with tc.tile_pool(name="pse", bufs=4, space="PSUM") as pse:
            for cb in range(N // P):
                pst = pse.tile([P, P], FP16, tag="tpose", name=f"tp_{cb}")
                nc.tensor.transpose(pst[:], colmax[:, cb * P : (cb + 1) * P],
                                    ident16[:])
                nc.vector.tensor_reduce(
                    mins_sb[:, NT + cb : NT + cb + 1], pst[:], axis=AX.X, op=ALU.max
                )

        nc.sync.dma_start(mins_out, mins_sb[:])


_CACHE = {}


def _get_program():
    if "nc" not in _CACHE:
        nc = bacc.Bacc(
            "TRN2",
            target_bir_lowering=False,
            debug=False,
            enable_asserts=True,
            num_devices=NCORES,
        )
        src = nc.dram_tensor("src", [N, D], FP32, kind="ExternalInput")
        tgt = nc.dram_tensor("tgt", [N, D], FP32, kind="ExternalInput")
        src_t = nc.dram_tensor("src_t", [D, N], FP32, kind="ExternalInput")
        tgt_t = nc.dram_tensor("tgt_t", [D, N], FP32, kind="ExternalInput")
        ident = nc.dram_tensor("ident", [P, P], FP16, kind="ExternalInput")
        aux = nc.dram_tensor("aux", [2, N], FP16, kind="ExternalInput")
        mins = nc.dram_tensor("mins", [P, 2 * NT], FP32, kind="ExternalOutput")
        with tile.TileContext(nc) as tc:
            _kernel_body(tc, src.ap(), tgt.ap(), src_t.ap(), tgt_t.ap(),
                         ident.ap(), aux.ap(), mins.ap())
        nc.compile()
        _CACHE["nc"] = nc
    return _CACHE["nc"]


def kernel(source: np.ndarray, target: np.ndarray) -> np.ndarray:
    global LAST_RESULTS
    import os

    source = np.ascontiguousarray(np.asarray(source, dtype=np.float32))
    target = np.ascontiguousarray(np.asarray(target, dtype=np.float32))
    assert source.shape == (B, N, D) and target.shape == (B, N, D)

    nc = _get_program()
    eye = np.eye(P, dtype=np.float16)
    aux = np.stack([np.ones(N, np.float16), np.zeros(N, np.float16)])
    in_maps = [
        {
            "src": source[b],
            "tgt": target[b],
            "src_t": np.ascontiguousarray(source[b].T),
            "tgt_t": np.ascontiguousarray(target[b].T),
            "ident": eye,
            "aux": aux,
        }
        for b in range(B)
    ]
    trace = os.environ.get("CHAMFER_TRACE", "0") == "1"
    tmpdir = os.environ.get("CHAMFER_TMPDIR") or None
    res = run_bass_kernel_spmd(
        nc, in_maps, core_ids=list(range(NCORES)), trace=trace, tmpdir=tmpdir
    )
    LAST_RESULTS = res

    # host epilogue: mins holds -0.5 * min d2 (as a max); clamp, scale, sqrt, mean
    loss = 0.0
    for b in range(B):
        m = res.results[b]["mins"].astype(np.float64)
        d2 = np.maximum(-2.0 * m, 0.0)
        dist = np.sqrt(d2)
        loss += dist[:, :NT].mean() + dist[:, NT:].mean()
    loss /= B
    return np.float32(loss)


# revision 22
# speedup vs baseline: 2.3142x; 1.2015x over previous
"""Chamfer distance loss kernel for Trainium2 (8 NeuronCores, batch-parallel).

Math: for each batch element, d2(i,j) = |s_i|^2 + |t_j|^2 - 2 s_i.t_j.
The whole distance computation folds into augmented matmuls:
  S_aug = [sx, sy, sz, -0.5|s|^2, 1]   (5 x 4096)
  T_aug = [tx, ty, tz, 1, -0.5|t|^2]   (5 x 4096)
so (S_aug)^T @ T_aug = s.t - 0.5|s|^2 - 0.5|t|^2 = -0.5 * d2.
min_j d2 = -2 * max_j(-0.5 d2): every reduction becomes a MAX.

fp32 matmuls are compiler-split ~4x (fp32-high emulation), so each augmented
matrix is kept as an fp16 hi/lo pair (x ~= hi + lo, 22-bit effective
mantissa). Each distance tile needs hi.hi + hi.lo + lo.hi (lo.lo ~ 2^-22 is
dropped); the two cross terms are fused into ONE K=10 matmul using stacked
operands [s_hi; s_lo] . [t_lo; t_hi], so each tile costs 2 matmuls (K only
loads weights, streaming time is set by the 512 moving columns).

Layout: s_cat [128, N] holds s_hi rows 0-4 and s_lo rows 5-9; t_cat holds
t_lo rows 0-4 and t_hi rows 5-9; both replicated at partition bases
{32, 64, 96} so quads of source tiles use separate 32-row PE groups.

Per core (one batch element):
  - PE: per source-tile quad, 16 fp16 matmuls into 4 two-bank PSUM tiles.
  - ACT: evacuates PSUM to per-source-tile fp16 SBUF tiles e16 [128, 4096]
    (the only PSUM reader).
  - DVE (fp16 SBUF at 2x mode): one tensor_max into colmax per source tile,
    a halving fold chain + reduce for each source tile's row max.
  - PE transposes colmax blocks; DVE reduce_max finishes target maxes.
  - The 4096+4096 per-point values (-0.5*min d2) are DMAed to DRAM.
Host: clamp/scale/sqrt/mean in numpy and average the 8 batch scalars.
"""

import sys

for _p in ("/opt/trn_rl_repo", "/root/.axon_site/_ro/trn_rl_repo"):
    if _p not in sys.path:
        sys.path.insert(0, _p)

import numpy as np

import concourse.bass as bass
import concourse.bacc as bacc
import concourse.tile as tile
from concourse import mybir
from concourse.bass_utils import run_bass_kernel_spmd

FP32 = mybir.dt.float32
FP16 = mybir.dt.float16
AX = mybir.AxisListType
ALU = mybir.AluOpType

B = 8          # batch == number of cores
N = 4096       # points per cloud
D = 3
P = 128        # partition tile (source tile size)
NT = N // P    # 32 source tiles
CH = 512       # one PSUM bank of fp32
GRP = 1024     # two banks per PSUM tile
NG = N // GRP  # 4 column groups
NQ = NT // 4   # 8 source-tile quads
NCORES = 8

LAST_RESULTS = None  # BassKernelResults of the most recent run (for test.py)


def _build_half(tc, pool, dram_nat, dram_t, aux, hi5, lo5, s2_row, ones_row,
                pfx, deng):
    """Write the fp16 hi part into hi5 [5, N] (must start at partition 0)
    and the lo part into lo5 [5, N] (same constraint) from dram_nat [N, 3]
    and dram_t [3, N] (host-transposed, pure layout).

    rows 0..2 = coords, s2_row = -0.5*|pt|^2, ones_row = 1.0 (hi) / 0.0 (lo).
    """
    nc = tc.nc
    # coords: load [3, N] fp32, split hi/lo
    ct32 = pool.tile([3, N], FP32, tag=f"ct32_{pfx}", name=f"ct32_{pfx}")
    deng.dma_start(ct32[:], dram_t)
    nc.scalar.copy(hi5[0:3, :], ct32[:])
    nc.vector.tensor_sub(lo5[0:3, :], ct32[:], hi5[0:3, :])

    # -0.5|pt|^2 in the wide layout: partition p holds points [32p, 32p+32)
    comb = pool.tile([P, 96], FP32, tag=f"comb_{pfx}", name=f"comb_{pfx}")
    deng.dma_start(comb[:], dram_nat.rearrange("(p a) d -> p (a d)", p=P))
    sq = pool.tile([P, 96], FP32, tag=f"sq_{pfx}", name=f"sq_{pfx}")
    nc.scalar.square(sq[:], comb[:])
    s2 = pool.tile([P, 32], FP32, tag=f"s2_{pfx}", name=f"s2_{pfx}")
    nc.vector.tensor_reduce(
        s2[:], sq[:].rearrange("p (a d) -> p a d", d=3), axis=AX.X, op=ALU.add
    )
    nc.vector.tensor_scalar_mul(s2[:], s2[:], -0.5)
    s2h = pool.tile([P, 32], FP16, tag=f"s2h_{pfx}", name=f"s2h_{pfx}")
    nc.vector.tensor_copy(s2h[:], s2[:])
    s2l = pool.tile([P, 32], FP16, tag=f"s2l_{pfx}", name=f"s2l_{pfx}")
    nc.vector.tensor_sub(s2l[:], s2[:], s2h[:])
    # scatter [128, 32] -> [1, N]: iteration order (p, a) matches j = 32p + a
    deng.dma_start(hi5[s2_row : s2_row + 1, :], s2h[:])
    deng.dma_start(lo5[s2_row : s2_row + 1, :], s2l[:])
    # ones row: 1.0 for the hi part, 0.0 for the lo part
    deng.dma_start(hi5[ones_row : ones_row + 1, :], aux[0:1, :])
    deng.dma_start(lo5[ones_row : ones_row + 1, :], aux[1:2, :])


def _kernel_body(tc, src, tgt, src_t, tgt_t, ident_dram, aux_dram, mins_out):
    nc = tc.nc
    with (
        tc.tile_pool(name="const", bufs=1) as const_pool,
        tc.tile_pool(name="aug", bufs=1) as aug_pool,
        tc.tile_pool(name="build", bufs=1) as build_pool,
        tc.tile_pool(name="acc", bufs=1) as acc_pool,
        tc.tile_pool(name="accq", bufs=2) as accq_pool,
        tc.tile_pool(name="fold", bufs=1) as fold_pool,
    ):
        ident16 = const_pool.tile([P, P], FP16)
        nc.sync.dma_start(ident16[:], ident_dram)
        aux = const_pool.tile([2, N], FP16)
        nc.sync.dma_start(aux[:], aux_dram)

        # s_cat:   rows 0-4 = s_hi aug, rows 5-9 = s_lo aug
        # t_cat_a: rows 0-4 = t_hi aug, rows 5-9 = t_lo aug  (-> hi.hi + lo.lo)
        # t_cat_b: rows 0-4 = t_lo aug, rows 5-9 = t_hi aug  (-> hi.lo + lo.hi)
        s_cat = aug_pool.tile([P, N], FP16, tag="s_cat")
        t_cat_a = aug_pool.tile([P, N], FP16, tag="t_cat_a")
        t_cat_b = aug_pool.tile([P, N], FP16, tag="t_cat_b")
        # compute-engine writes can only start at partitions {0,32,64,96},
        # so parts destined for rows 5-9 are built at base 0 and DMAed up
        s_lo5 = aug_pool.tile([5, N], FP16, tag="s_lo5")
        t_lo5 = aug_pool.tile([5, N], FP16, tag="t_lo5")

        _build_half(tc, build_pool, src, src_t, aux, s_cat[0:5, :], s_lo5[:],
                    s2_row=3, ones_row=4, pfx="s", deng=nc.sync)
        _build_half(tc, build_pool, tgt, tgt_t, aux, t_cat_a[0:5, :], t_lo5[:],
                    s2_row=4, ones_row=3, pfx="t", deng=nc.scalar)
        nc.sync.dma_start(s_cat[5:10, :], s_lo5[:])
        nc.scalar.dma_start(t_cat_a[5:10, :], t_lo5[:])
        nc.sync.dma_start(t_cat_b[0:5, :], t_lo5[:])
        nc.scalar.dma_start(t_cat_b[5:10, :], t_cat_a[0:5, :])

        # replicate rows 0..9 at partition bases 32/64/96 for row-group use
        engs = [nc.sync, nc.scalar]
        for ti, t in enumerate((s_cat, t_cat_a, t_cat_b)):
            for ri, base in enumerate((32, 64, 96)):
                engs[(ti + ri) % 2].dma_start(t[base : base + 10, :], t[0:10, :])

        # accumulators
        colmax = acc_pool.tile([P, N], FP16, tag="colmax")      # [src_part, tgt_col]
        mins_sb = acc_pool.tile([P, 2 * NT], FP32, tag="mins")

        with tc.tile_pool(name="psum", bufs=1, space="PSUM") as psum_pool:
            for iq in range(NQ):
                e16s = [
                    accq_pool.tile([P, N], FP16, tag=f"e16_{q}", name=f"e16_{iq}_{q}")
                    for q in range(4)
                ]
                for g in range(NG):
                    pss = [
                        psum_pool.tile([P, GRP], FP32, tag=f"d2_{q}",
                                       name=f"d2_{iq}_{g}_{q}")
                        for q in range(4)
                    ]
                    for j in range(2):
                        c = 2 * g + j
                        for q in range(4):
                            it = iq * 4 + q
                            b = 32 * q
                            lhsT = s_cat[b : b + 10, it * P : (it + 1) * P]
                            # hi.hi + lo.lo
                            nc.tensor.matmul(
                                pss[q][:, j * CH : (j + 1) * CH],
                                lhsT,
                                t_cat_a[b : b + 10, c * CH : (c + 1) * CH],
                                start=True,
                                stop=False,
                                tile_position=(b, 0),
                            )
                            # hi.lo + lo.hi
                            nc.tensor.matmul(
                                pss[q][:, j * CH : (j + 1) * CH],
                                lhsT,
                                t_cat_b[b : b + 10, c * CH : (c + 1) * CH],
                                start=False,
                                stop=True,
                                tile_position=(b, 0),
                            )
                    for q in range(4):
                        nc.scalar.copy(
                            e16s[q][:, g * GRP : (g + 1) * GRP], pss[q][:]
                        )
                for q in range(4):
                    e16 = e16s[q]
                    # column/target fold: one big fp16 2x op per source tile
                    if iq == 0 and q == 0:
                        nc.vector.tensor_copy(colmax[:], e16[:])
                    else:
                        nc.vector.tensor_max(colmax[:], colmax[:], e16[:])
                    # row/source max: halving folds then one small reduce
                    f1 = fold_pool.tile([P, N // 2], FP16, tag=f"f1_{q}",
                                        name=f"f1_{iq}_{q}")
                    nc.vector.tensor_max(f1[:], e16[:, 0 : N // 2],
                                         e16[:, N // 2 : N])
                    f2 = fold_pool.tile([P, N // 4], FP16, tag=f"f2_{q}",
                                        name=f"f2_{iq}_{q}")
                    nc.vector.tensor_max(f2[:], f1[:, 0 : N // 4],
                                         f1[:, N // 4 : N // 2])
                    f3 = fold_pool.tile([P, N // 8], FP16, tag=f"f3_{q}",
                                        name=f"f3_{iq}_{q}")
                    nc.vector.tensor_max(f3[:], f2[:, 0 : N // 8],
                                         f2[:, N // 8 : N // 4])
                    it = iq * 4 + q
                    nc.vector.tensor_reduce(
                        mins_sb[:, it : it + 1], f3[:], axis=AX.X, op=ALU.max
                    )

        # finish columns: transpose 128-blocks, reduce over former partitions
        with tc.tile_pool(name="pse", bufs=4, space="PSUM") as pse:
            for cb in range(N // P):
                pst = pse.tile([P, P], FP16, tag="tpose", name=f"tp_{cb}")
                nc.tensor.transpose(pst[:], colmax[:, cb * P : (cb + 1) * P],
                                    ident16[:])
                nc.vector.tensor_reduce(
                    mins_sb[:, NT + cb : NT + cb + 1], pst[:], axis=AX.X, op=ALU.max
                )

        nc.sync.dma_start(mins_out, mins_sb[:])


_CACHE = {}


def _get_program():
    if "nc" not in _CACHE:
        nc = bacc.Bacc(
            "TRN2",
            target_bir_lowering=False,
            debug=False,
            enable_asserts=True,
            num_devices=NCORES,
        )
        src = nc.dram_tensor("src", [N, D], FP32, kind="ExternalInput")
        tgt = nc.dram_tensor("tgt", [N, D], FP32, kind="ExternalInput")
        src_t = nc.dram_tensor("src_t", [D, N], FP32, kind="ExternalInput")
        tgt_t = nc.dram_tensor("tgt_t", [D, N], FP32, kind="ExternalInput")
        ident = nc.dram_tensor("ident", [P, P], FP16, kind="ExternalInput")
        aux = nc.dram_tensor("aux", [2, N], FP16, kind="ExternalInput")
        mins = nc.dram_tensor("mins", [P, 2 * NT], FP32, kind="ExternalOutput")
        with tile.TileContext(nc) as tc:
            _kernel_body(tc, src.ap(), tgt.ap(), src_t.ap(), tgt_t.ap(),
                         ident.ap(), aux.ap(), mins.ap())
        nc.compile()
        _CACHE["nc"] = nc
    return _CACHE["nc"]


def kernel(source: np.ndarray, target: np.ndarray) -> np.ndarray:
    global LAST_RESULTS
    import os

    source = np.ascontiguousarray(np.asarray(source, dtype=np.float32))
    target = np.ascontiguousarray(np.asarray(target, dtype=np.float32))
    assert source.shape == (B, N, D) and target.shape == (B, N, D)

    nc = _get_program()
    eye = np.eye(P, dtype=np.float16)
    aux = np.stack([np.ones(N, np.float16), np.zeros(N, np.float16)])
    in_maps = [
        {
            "src": source[b],
            "tgt": target[b],
            "src_t": np.ascontiguousarray(source[b].T),
            "tgt_t": np.ascontiguousarray(target[b].T),
            "ident": eye,
            "aux": aux,
        }
        for b in range(B)
    ]
    trace = os.environ.get("CHAMFER_TRACE", "0") == "1"
    tmpdir = os.environ.get("CHAMFER_TMPDIR") or None
    res = run_bass_kernel_spmd(
        nc, in_maps, core_ids=list(range(NCORES)), trace=trace, tmpdir=tmpdir
    )
    LAST_RESULTS = res

    # host epilogue: mins holds -0.5 * min d2 (as a max); clamp, scale, sqrt, mean
    loss = 0.0
    for b in range(B):
        m = res.results[b]["mins"].astype(np.float64)
        d2 = np.maximum(-2.0 * m, 0.0)
        dist = np.sqrt(d2)
        loss += dist[:, :NT].mean() + dist[:, NT:].mean()
    loss /= B
    return np.float32(loss)
